# revision 2
# baseline (speedup 1.0000x reference)
"""Trainium2 Bass kernel for CustomQuantLinear (int8-range weight quant linear).

out[b,s,o] = sum_i x[b,s,i] * (w[o,i] - zp[o]) * scale[o] + bias[o]

Sharding: column-parallel over out_features across 8 NeuronCores
(1376 features per core), x replicated.

Device strategy per core:
  - Host stages the weight shard as ready-to-use fp16 w_rec tiles
    [4096k x 1376n] ((w - zp) * scale in fp16 — bit-identical to the
    on-chip DVE dequant this replaces: w, zp are fp16-exact ints, scale
    is fp16, single rounding on the product).
  - Stream x as pre-tiled [128k x 4096(m-major)] fp16 slabs (host does
    the layout permute + f32->f16 staging cast) and use 128x128 x-tiles
    as the stationary matmul operand.
  - Accumulate psum[m=128, nf<=512] over 32 k-chunks on the PE at
    fp16 rate (1 cycle/row), add bias on DVE, DMA out in natural
    [m, n] layout.
  - DMA traffic is split across both HWDGE queues so the PE never
    starves: x slabs on qAct (64 MB), outputs on qSP (45 MB), w_rec
    load alternating over both; the first 3 x slabs are issued ahead
    of the w_rec block so m-tile 0 can start immediately.

PE floor is 64*32*1376 = 2,818,048 MM row-cycles/core: 1.41 ms at the
2.0 GHz sustained clock, 1.17 ms when the part boosts to 2.4 GHz.
"""

import os
import sys

import numpy as np

for _p in ("/opt/trn_rl_repo",):
    if _p not in sys.path and os.path.isdir(_p):
        sys.path.append(_p)

import concourse.bass as bass
import concourse.mybir as mybir
import concourse.tile as tile
from concourse.bass_utils import run_bass_kernel_spmd
from concourse.vector_clock import ScopedClock

N_CORES = 8
B, S, IN, OUT = 4, 2048, 4096, 11008
M = B * S                  # 8192 rows
N_SHARD = OUT // N_CORES   # 1376 out-features per core
P = 128
NMI = M // P               # 64 m-tiles
NKC = IN // P              # 32 k-chunks
NF_CHUNKS = (512, 512, 352)

f32 = mybir.dt.float32
f16 = mybir.dt.float16


def _patch_tile_drain():
    """This walrus build rejects >1 sem-wait on an InstDrain
    (setupSyncWait<...CTRL_NO_STRUCT>: "Too many sync wait commands").
    Split the Tile tail-drain into one single-wait drain per semaphore."""
    if getattr(tile.TileContext, "_drain_patch_applied", False):
        return

    def _drain_and_barrier(self, tick_clock, wait_clock):
        drain_inst = self.nc.sync.drain()
        wait_clock.add_sem_waits(
            drain_inst.ins, ScopedClock({None: tick_clock.global_clock})
        )
        si = drain_inst.ins.sync_info
        waits = list(si.on_wait) if si is not None else []
        if len(waits) > 1:
            drain_inst.ins.sync_info = mybir.SyncInfo(
                on_wait=[waits[0]], on_update=[]
            )
            for w in waits[1:]:
                d2 = self.nc.sync.drain()
                d2.ins.sync_info = mybir.SyncInfo(on_wait=[w], on_update=[])

        self.nc.all_engine_barrier()
        assert self.sems is not None
        popped = self.nc._tile_sem_poison_stack.pop()
        assert popped is self._sem_poison
        self.nc.clear_and_free_semaphores(list(self.sems.allocated().values()))
        self.nc.all_engine_barrier()

    tile.TileContext._drain_and_barrier = _drain_and_barrier
    tile.TileContext._drain_patch_applied = True


def _split_multi_wait_instructions(nc):
    """This walrus build allows at most ONE sem-wait per instruction
    (setupSyncWait: "Too many sync wait commands"). Move extra waits onto
    same-engine NoOps inserted right before the instruction — the engine
    executes sequentially, so blocking on each sem in turn is equivalent."""
    counter = 0
    for fn in nc.m.functions:
        for bb in fn.blocks:
            new = []
            changed = False
            for inst in bb.instructions:
                si = inst.sync_info
                waits = list(si.on_wait) if si is not None else []
                if len(waits) > 1:
                    changed = True
                    for w in waits[:-1]:
                        counter += 1
                        nop = mybir.InstNoOp(
                            name=f"waitsplit-{counter}", ins=[], outs=[]
                        )
                        nop.engine = inst.engine
                        nop.sync_info = mybir.SyncInfo(on_wait=[w], on_update=[])
                        new.append(nop)
                    inst.sync_info = mybir.SyncInfo(
                        on_wait=[waits[-1]], on_update=list(si.on_update)
                    )
                new.append(inst)
            if changed:
                bb.instructions = new
    return counter


def build_nc(
    nmi=NMI,
    nkc=NKC,
    n_shard=N_SHARD,
    nf_chunks=NF_CHUNKS,
    repeat=1,
    x_prefetch=3,
):
    """Build the per-core Bass program (SPMD; per-core data differs).

    repeat>1 wraps the whole body in a hardware For_i loop (idempotent
    re-execution) — a timing instrument to cancel host dispatch overhead.
    """
    _patch_tile_drain()
    k = nkc * P
    nc = bass.Bass()

    x_in = nc.dram_tensor("x3", [nmi, P, k], f16, kind="ExternalInput")
    w_in = nc.dram_tensor("wt", [k, n_shard], f16, kind="ExternalInput")
    b_in = nc.dram_tensor("biasb", [P, n_shard], f32, kind="ExternalInput")
    out = nc.dram_tensor("out", [nmi * P, n_shard], f32, kind="ExternalOutput")

    from contextlib import ExitStack

    with tile.TileContext(nc) as tc:
        with (
            tc.tile_pool(name="const", bufs=1) as constp,
            tc.tile_pool(name="wrec", bufs=nkc) as wrecp,
            tc.tile_pool(name="xf16", bufs=3) as xf16p,
            tc.tile_pool(name="psum", bufs=2, space="PSUM") as psump,
            tc.tile_pool(name="outs", bufs=3) as outp,
            ExitStack() as loop_ctx,
        ):
            if repeat > 1:
                loop_ctx.enter_context(tc.For_i(0, repeat, 1))

            # First x slabs go out on qAct ahead of the w_rec block so
            # m-tile 0's MMs can start as soon as slab 0 lands.
            xtiles = []
            for mi in range(min(x_prefetch, nmi)):
                xf16 = xf16p.tile([P, k], f16)
                nc.scalar.dma_start(xf16[:], x_in[mi])
                xtiles.append(xf16)

            bias_b = constp.tile([P, n_shard], f32, tag="bias")
            nc.sync.dma_start(bias_b[:], b_in[:])

            # Ready-dequantized fp16 weights, alternating both HWDGE queues.
            wrecs = []
            for kc in range(nkc):
                wr = wrecp.tile([P, n_shard], f16, tag="wr")
                eng = nc.sync if kc % 2 == 0 else nc.scalar
                eng.dma_start(wr[:], w_in[kc * P : (kc + 1) * P, :])
                wrecs.append(wr)

            nf_offs = [sum(nf_chunks[:j]) for j in range(len(nf_chunks))]
            for mi in range(nmi):
                if mi < len(xtiles):
                    xf16 = xtiles[mi]
                else:
                    xf16 = xf16p.tile([P, k], f16)
                    nc.scalar.dma_start(xf16[:], x_in[mi])

                psums = [
                    psump.tile([P, nf], f32, tag=f"ps{j}", name=f"ps{j}")
                    for j, nf in enumerate(nf_chunks)
                ]
                for kc in range(nkc):
                    for j, nf in enumerate(nf_chunks):
                        nfo = nf_offs[j]
                        nc.tensor.matmul(
                            psums[j][:],
                            xf16[:, kc * P : (kc + 1) * P],
                            wrecs[kc][:, nfo : nfo + nf],
                            start=(kc == 0),
                            stop=(kc == nkc - 1),
                        )

                for j, nf in enumerate(nf_chunks):
                    nfo = nf_offs[j]
                    ot = outp.tile([P, nf], f32, tag=f"o{j}", name=f"o{j}")
                    nc.vector.tensor_tensor(
                        ot[:],
                        psums[j][:],
                        bias_b[:, nfo : nfo + nf],
                        op=mybir.AluOpType.add,
                    )
                    nc.sync.dma_start(
                        out[mi * P : (mi + 1) * P, nfo : nfo + nf], ot[:]
                    )

    return nc


# flags actually used by kernel(); calibration scripts override per-build
BEST_CONFIG = dict(x_prefetch=3)


def _prep_inputs(x, weight, scale, zp, bias):
    """Host-side shard/layout prep (layout permute + dtype staging +
    fp16 weight recovery, same rounding as the on-chip dequant)."""
    x = np.asarray(x, dtype=np.float32)
    weight = np.asarray(weight)
    scale = np.asarray(scale)
    zp = np.asarray(zp)
    bias = np.asarray(bias, dtype=np.float32)

    # [mi, p(k%128), kc*128+j(m%128)] so each m-tile is one contiguous slab
    # whose kc-th 128-column block is the stationary lhsT [k, m] tile.
    # f16 staging applies the same RTNE rounding an on-chip cast would.
    X = np.ascontiguousarray(
        x.reshape(NMI, P, NKC, P).transpose(0, 3, 2, 1).reshape(NMI, P, NKC * P)
    ).astype(np.float16)

    in_maps = []
    for c in range(N_CORES):
        sl = slice(c * N_SHARD, (c + 1) * N_SHARD)
        ws = weight[sl].astype(np.float16)       # [1376, 4096], ints: exact
        zps = zp[sl, 0].astype(np.float16)       # exact
        scs = scale[sl, 0].astype(np.float16)    # scale is already fp16
        # fp16 arithmetic throughout = bit-identical to DVE f16 dequant
        wrec = (ws - zps[:, None]) * scs[:, None]
        bs = bias[sl].astype(np.float32)
        in_maps.append(
            {
                "x3": X,
                "wt": np.ascontiguousarray(wrec.T),
                "biasb": np.ascontiguousarray(
                    np.broadcast_to(bs[None, :], (P, N_SHARD))
                ),
            }
        )
    return in_maps


def run(inputs, trace=False):
    """Returns (full_output [4,2048,11008] f32, BassKernelResults)."""
    in_maps = _prep_inputs(**inputs)
    nc = build_nc(**BEST_CONFIG)
    _split_multi_wait_instructions(nc)
    res = run_bass_kernel_spmd(nc, in_maps, list(range(N_CORES)), trace=trace)
    shards = [res.results[i]["out"] for i in range(N_CORES)]
    full = np.concatenate(shards, axis=1).reshape(B, S, OUT).astype(np.float32)
    return full, res


def kernel(**inputs) -> np.ndarray:
    out, _ = run(inputs, trace=False)
    return out


# revision 8
# speedup vs baseline: 1.1349x; 1.1349x over previous
"""Trainium2 Bass kernel for CustomQuantLinear (int8-range weight quant linear).

out[b,s,o] = sum_i x[b,s,i] * (w[o,i] - zp[o]) * scale[o] + bias[o]

Sharding: column-parallel over out_features across 8 NeuronCores
(1376 features per core), x replicated.

Device strategy per core:
  - Host stages the weight shard as ready-to-use fp16 w_rec tiles
    [4096k x 1376n] ((w - zp) * scale in fp16 — bit-identical to the
    on-chip DVE dequant this replaces: w, zp are fp16-exact ints, scale
    is fp16, single rounding on the product).
  - Stream x as pre-tiled [128k x 4096(m-major)] fp16 slabs (host does
    the layout permute + f32->f16 staging cast) and use 128x128 x-tiles
    as the stationary matmul operand.
  - Accumulate psum[m=128, nf<=512] over 32 k-chunks on the PE at
    fp16 rate (1 cycle/row), add bias on DVE, DMA out in natural
    [m, n] layout.
  - DMA traffic is split across both HWDGE queues so the PE never
    starves: x slabs on qAct (64 MB), outputs on qSP (45 MB), w_rec
    load alternating over both; the first 3 x slabs are issued ahead
    of the w_rec block so m-tile 0 can start immediately.

PE floor is 64*32*1376 = 2,818,048 MM row-cycles/core: 1.41 ms at the
2.0 GHz sustained clock, 1.17 ms when the part boosts to 2.4 GHz.
"""

import os
import sys

import numpy as np

for _p in ("/opt/trn_rl_repo",):
    if _p not in sys.path and os.path.isdir(_p):
        sys.path.append(_p)

import concourse.bass as bass
import concourse.mybir as mybir
import concourse.tile as tile
from concourse.bass_utils import run_bass_kernel_spmd
from concourse.vector_clock import ScopedClock

N_CORES = 8
B, S, IN, OUT = 4, 2048, 4096, 11008
M = B * S                  # 8192 rows
N_SHARD = OUT // N_CORES   # 1376 out-features per core
P = 128
NMI = M // P               # 64 m-tiles
NKC = IN // P              # 32 k-chunks
NF_CHUNKS = (512, 512, 352)

f32 = mybir.dt.float32
f16 = mybir.dt.float16


def _patch_tile_drain():
    """This walrus build rejects >1 sem-wait on an InstDrain
    (setupSyncWait<...CTRL_NO_STRUCT>: "Too many sync wait commands").
    Split the Tile tail-drain into one single-wait drain per semaphore."""
    if getattr(tile.TileContext, "_drain_patch_applied", False):
        return

    def _drain_and_barrier(self, tick_clock, wait_clock):
        drain_inst = self.nc.sync.drain()
        wait_clock.add_sem_waits(
            drain_inst.ins, ScopedClock({None: tick_clock.global_clock})
        )
        si = drain_inst.ins.sync_info
        waits = list(si.on_wait) if si is not None else []
        if len(waits) > 1:
            drain_inst.ins.sync_info = mybir.SyncInfo(
                on_wait=[waits[0]], on_update=[]
            )
            for w in waits[1:]:
                d2 = self.nc.sync.drain()
                d2.ins.sync_info = mybir.SyncInfo(on_wait=[w], on_update=[])

        self.nc.all_engine_barrier()
        assert self.sems is not None
        popped = self.nc._tile_sem_poison_stack.pop()
        assert popped is self._sem_poison
        self.nc.clear_and_free_semaphores(list(self.sems.allocated().values()))
        self.nc.all_engine_barrier()

    tile.TileContext._drain_and_barrier = _drain_and_barrier
    tile.TileContext._drain_patch_applied = True


def _split_multi_wait_instructions(nc):
    """This walrus build allows at most ONE sem-wait per instruction
    (setupSyncWait: "Too many sync wait commands"). Move extra waits onto
    same-engine NoOps inserted right before the instruction — the engine
    executes sequentially, so blocking on each sem in turn is equivalent."""
    counter = 0
    for fn in nc.m.functions:
        for bb in fn.blocks:
            new = []
            changed = False
            for inst in bb.instructions:
                si = inst.sync_info
                waits = list(si.on_wait) if si is not None else []
                if len(waits) > 1:
                    changed = True
                    for w in waits[:-1]:
                        counter += 1
                        nop = mybir.InstNoOp(
                            name=f"waitsplit-{counter}", ins=[], outs=[]
                        )
                        nop.engine = inst.engine
                        nop.sync_info = mybir.SyncInfo(on_wait=[w], on_update=[])
                        new.append(nop)
                    inst.sync_info = mybir.SyncInfo(
                        on_wait=[waits[-1]], on_update=list(si.on_update)
                    )
                new.append(inst)
            if changed:
                bb.instructions = new
    return counter


def build_nc(
    nmi=NMI,
    nkc=NKC,
    n_shard=N_SHARD,
    nf_chunks=NF_CHUNKS,
    repeat=1,
    x_prefetch=3,
    dma_scheme="split",
    xf16_bufs=3,
    wrec_bufs=None,
):
    """Build the per-core Bass program (SPMD; per-core data differs).

    repeat>1 wraps the whole body in a hardware For_i loop (idempotent
    re-execution) — a timing instrument to cancel host dispatch overhead.
    """
    _patch_tile_drain()
    k = nkc * P
    nc = bass.Bass()

    x_in = nc.dram_tensor("x3", [nmi, P, k], f16, kind="ExternalInput")
    w_in = nc.dram_tensor("wt", [k, n_shard], f16, kind="ExternalInput")
    b_in = nc.dram_tensor("biasb", [P, n_shard], f32, kind="ExternalInput")
    out = nc.dram_tensor("out", [nmi * P, n_shard], f32, kind="ExternalOutput")

    from contextlib import ExitStack

    with tile.TileContext(nc) as tc:
        with (
            tc.tile_pool(name="const", bufs=1) as constp,
            tc.tile_pool(name="wrec", bufs=wrec_bufs or nkc) as wrecp,
            tc.tile_pool(name="xf16", bufs=xf16_bufs) as xf16p,
            tc.tile_pool(name="psum", bufs=2, space="PSUM") as psump,
            tc.tile_pool(name="outs", bufs=3) as outp,
            ExitStack() as loop_ctx,
        ):
            if repeat > 1:
                loop_ctx.enter_context(tc.For_i(0, repeat, 1))

            if dma_scheme == "sync_only":
                x_eng, out_eng, w_engs = nc.sync, nc.sync, (nc.sync, nc.sync)
            elif dma_scheme == "wsync":
                x_eng, out_eng, w_engs = nc.scalar, nc.sync, (nc.sync, nc.sync)
            else:  # split
                x_eng, out_eng, w_engs = nc.scalar, nc.sync, (nc.sync, nc.scalar)

            # First x slabs go out on qAct ahead of the w_rec block so
            # m-tile 0's MMs can start as soon as slab 0 lands.
            xtiles = []
            for mi in range(min(x_prefetch, nmi)):
                xf16 = xf16p.tile([P, k], f16)
                x_eng.dma_start(xf16[:], x_in[mi])
                xtiles.append(xf16)

            bias_b = constp.tile([P, n_shard], f32, tag="bias")
            nc.sync.dma_start(bias_b[:], b_in[:])

            # Ready-dequantized fp16 weights, alternating both HWDGE queues.
            wrecs = []
            for kc in range(nkc):
                wr = wrecp.tile([P, n_shard], f16, tag="wr")
                eng = w_engs[kc % 2]
                eng.dma_start(wr[:], w_in[kc * P : (kc + 1) * P, :])
                wrecs.append(wr)

            nf_offs = [sum(nf_chunks[:j]) for j in range(len(nf_chunks))]
            for mi in range(nmi):
                if mi < len(xtiles):
                    xf16 = xtiles[mi]
                else:
                    xf16 = xf16p.tile([P, k], f16)
                    x_eng.dma_start(xf16[:], x_in[mi])

                psums = [
                    psump.tile([P, nf], f32, tag=f"ps{j}", name=f"ps{j}")
                    for j, nf in enumerate(nf_chunks)
                ]
                for kc in range(nkc):
                    for j, nf in enumerate(nf_chunks):
                        nfo = nf_offs[j]
                        nc.tensor.matmul(
                            psums[j][:],
                            xf16[:, kc * P : (kc + 1) * P],
                            wrecs[kc][:, nfo : nfo + nf],
                            start=(kc == 0),
                            stop=(kc == nkc - 1),
                        )

                for j, nf in enumerate(nf_chunks):
                    nfo = nf_offs[j]
                    ot = outp.tile([P, nf], f32, tag=f"o{j}", name=f"o{j}")
                    nc.vector.tensor_tensor(
                        ot[:],
                        psums[j][:],
                        bias_b[:, nfo : nfo + nf],
                        op=mybir.AluOpType.add,
                    )
                    out_eng.dma_start(
                        out[mi * P : (mi + 1) * P, nfo : nfo + nf], ot[:]
                    )

    return nc


# flags actually used by kernel(); calibration scripts override per-build
BEST_CONFIG = dict(x_prefetch=3)


def _prep_inputs(x, weight, scale, zp, bias):
    """Host-side shard/layout prep (layout permute + dtype staging +
    fp16 weight recovery, same rounding as the on-chip dequant)."""
    x = np.asarray(x, dtype=np.float32)
    weight = np.asarray(weight)
    scale = np.asarray(scale)
    zp = np.asarray(zp)
    bias = np.asarray(bias, dtype=np.float32)

    # [mi, p(k%128), kc*128+j(m%128)] so each m-tile is one contiguous slab
    # whose kc-th 128-column block is the stationary lhsT [k, m] tile.
    # f16 staging applies the same RTNE rounding an on-chip cast would.
    X = np.ascontiguousarray(
        x.reshape(NMI, P, NKC, P).transpose(0, 3, 2, 1).reshape(NMI, P, NKC * P)
    ).astype(np.float16)

    in_maps = []
    for c in range(N_CORES):
        sl = slice(c * N_SHARD, (c + 1) * N_SHARD)
        ws = weight[sl].astype(np.float16)       # [1376, 4096], ints: exact
        zps = zp[sl, 0].astype(np.float16)       # exact
        scs = scale[sl, 0].astype(np.float16)    # scale is already fp16
        # fp16 arithmetic throughout = bit-identical to DVE f16 dequant
        wrec = (ws - zps[:, None]) * scs[:, None]
        bs = bias[sl].astype(np.float32)
        in_maps.append(
            {
                "x3": X,
                "wt": np.ascontiguousarray(wrec.T),
                "biasb": np.ascontiguousarray(
                    np.broadcast_to(bs[None, :], (P, N_SHARD))
                ),
            }
        )
    return in_maps


def run(inputs, trace=False):
    """Returns (full_output [4,2048,11008] f32, BassKernelResults)."""
    in_maps = _prep_inputs(**inputs)
    nc = build_nc(**BEST_CONFIG)
    _split_multi_wait_instructions(nc)
    res = run_bass_kernel_spmd(nc, in_maps, list(range(N_CORES)), trace=trace)
    shards = [res.results[i]["out"] for i in range(N_CORES)]
    full = np.concatenate(shards, axis=1).reshape(B, S, OUT).astype(np.float32)
    return full, res


def kernel(**inputs) -> np.ndarray:
    out, _ = run(inputs, trace=False)
    return out
